# revision 11
# baseline (speedup 1.0000x reference)
"""Multi-head attention (B*H=64, S=2048, D=64) on 8 Trainium2 cores.

Sharding: 64 heads -> 8 per core (head-parallel, no communication).

Per-core kernel (heads processed in pairs A/B stacked on SBUF partition
halves 0:64 / 64:128):
  - prep (all pairs upfront): q/k are DMA'd with a 32x32-block-permuted
    access pattern, then a single DVE 32x32 block-transpose per tensor
    yields stacked Q^T/K^T [128(dA|dB), 2048] in natural q order; the
    fp16 rounding copy runs on GPSIMD (idle engine) to keep DVE free for
    exp.  V is loaded naturally and cast to bf16 (no ones column).
  - main loop per pair x (q-chunk 512) x (16 k-tiles of 128):
      S^T[k,q] = K Q^T   -- two row-packed fp16 matmuls
                            (tile_position (0,0)/(64,0)), concurrent on PE
      P^T = exp(S^T)     -- bf16; exact exp on ACT for some k-tiles,
                            Schraudolph int16 bit-trick on DVE for the rest
      O^T[d,q] += V^T P^T -- two col-packed bf16 matmuls, M=64 each at
                            tile_position (0,0)/(0,64): heads A/B land on
                            PSUM partitions 0:64 / 64:128 of ONE bank and
                            stream concurrently (half the PE time of the
                            old M=65 ones-column scheme)
      Z[q]    += 1^T P^T -- two col-packed M=1 ones matmuls into a
                            second bank (rows 0 / 64), also concurrent
  - epilogue per (pair, q-chunk): ACT-copy O^T and Z to SBUF, PE-transpose
    back to [q, d] (+tiny [q, 2] Z transposes into the same PSUM tile),
    DVE reciprocal of Z^T, broadcast-multiply, DMA out.
"""

import os

import numpy as np

import concourse.bass as bass
import concourse.mybir as mybir
import concourse.tile as tile
from concourse import bacc
from concourse.bass_utils import run_bass_kernel_spmd
from concourse.masks import make_identity

B, S, D = 64, 2048, 64
NCORES = 8
H = B // NCORES  # heads per core
P = 128  # partitions
KT = S // P  # 16 k-tiles
QC = 512  # q-chunk
NQC = S // QC  # 4 q-chunks
NPAIR = H // 2  # head pairs per core
PIPE = 3  # k-tiles of QK^T in flight ahead of exp/PV

F32 = mybir.dt.float32
BF16 = mybir.dt.bfloat16
I16 = mybir.dt.int16
F16 = mybir.dt.float16

# Number of k-tiles (of 16) whose exp runs on DVE via the Schraudolph bit
# trick (approximate, ~2% rms per weight); the rest run exact exp on ACT.
# kt 0 and 1 are always DVE: at each q-chunk boundary ACT is busy with the
# PSUM->SBUF eviction copies of the previous chunk, so the first s-tiles
# must drain on DVE to keep the PE fed.
DVE_EXP_KT = int(os.environ.get("BASS_ATTN_DVE_EXP_KT", "7"))
_DVE_KTS = {0, 1}
n_rest = max(0, DVE_EXP_KT - 2)
if n_rest > 0:
    _DVE_KTS |= {2 + round((i + 0.5) * 14 / n_rest) % 14 for i in range(n_rest)}

# Schraudolph constants for bf16 exp via int16 bit pattern:
#   i = round_int16(x * 2^7/ln2 + b);  exp(x) ~= bitcast_bf16(i)
# b calibrated for round-to-nearest convert (max rel err ~3.3%).
_SCH_A = float(128.0 / np.log(2.0))
_SCH_B = float(os.environ.get("BASS_ATTN_SCH_B", "16250.5"))

# Engine for the prep fp16/bf16 rounding copies: gpsimd (default) or vector.
GPSIMD_CAST = os.environ.get("BASS_ATTN_GPSIMD_CAST", "1") == "1"
# Epilogue normalize: DVE broadcast tensor_tensor (default) or 8x ACT mul.
NORM_ACT = os.environ.get("BASS_ATTN_NORM_ACT", "0") == "1"


def build_attention_nc() -> bass.Bass:
    nc = bacc.Bacc()
    q_d = nc.declare_dram_parameter("q", [H, S, D], F32, isOutput=False)
    k_d = nc.declare_dram_parameter("k", [H, S, D], F32, isOutput=False)
    v_d = nc.declare_dram_parameter("v", [H, S, D], F32, isOutput=False)
    o_d = nc.declare_dram_parameter("out", [H, S, D], F32, isOutput=True)

    # 32x32-block-permuted views for the transpose loads:
    #   staging[32a + i, 32b + j] = x[32b + i, 32a + j]
    q_bp = q_d.rearrange("h (b i) (a j) -> h a i b j", i=32, j=32)
    k_bp = k_d.rearrange("h (b i) (a j) -> h a i b j", i=32, j=32)
    # natural views: row = t*128 + m (k index), row = g*512 + c*128 + p (q)
    v_v = v_d.rearrange("h (t p) d -> h p t d", p=P)
    o_v = o_d.rearrange("h (g c p) d -> h p g c d", c=4, p=P)

    cast_eng = None  # set inside context

    with tile.TileContext(nc) as tc:
        with (
            tc.tile_pool(name="consts", bufs=1) as consts,
            tc.tile_pool(name="stage", bufs=4) as stage,
            tc.tile_pool(name="qk_t", bufs=16) as qkt_pool,
            tc.tile_pool(name="vpool", bufs=4) as vpool,
            tc.tile_pool(name="ppool", bufs=8) as ppool,
            tc.tile_pool(name="osb", bufs=2) as osb_pool,
            tc.tile_pool(name="zsb", bufs=2) as zsb_pool,
            tc.tile_pool(name="outsb", bufs=4) as outsb_pool,
            tc.tile_pool(name="rz", bufs=4) as rz_pool,
            tc.tile_pool(name="spsum", bufs=PIPE, space="PSUM") as spsum,
            tc.tile_pool(name="pvp", bufs=1, space="PSUM") as pv_pool,
            tc.tile_pool(name="zp", bufs=1, space="PSUM") as z_pool,
        ):
            pass

            ident = consts.tile([P, P], F32)
            make_identity(nc, ident[:])
            ident33 = consts.tile([33, 33], F32)
            make_identity(nc, ident33[:])
            ones_w = consts.tile([P, 1], BF16)
            nc.vector.memset(ones_w[:], 1.0)

            # ---------------- prep (all pairs, emitted upfront) ----------------
            # K first (the kt loop sweeps all K chunks before moving to
            # the next q chunk), per-512-column chunks for fine-grained
            # pipelining of DMA -> block-transpose -> round.  Pool buffering
            # (qkT bufs=16 = 2 pairs) throttles execution so pair p+1's prep
            # overlaps pair p's main loop.
            qkt_all = []
            v_all = []
            for pair in range(NPAIR):
                h_a, h_b = 2 * pair, 2 * pair + 1
                # pair 0's prep is on the critical path at kernel start (no
                # main-loop work to hide behind): use the faster DVE for its
                # rounding casts; later pairs go to the otherwise-idle GPSIMD.
                cast_eng = (
                    nc.gpsimd if (GPSIMD_CAST and pair > 0) else nc.vector
                )
                qkt = {"q": [], "k": []}
                for fc in range(4):
                    for name, src in (("k", k_bp), ("q", q_bp)):
                        st = stage.tile([P, QC], F32, tag="stage")
                        for hh, pb in ((h_a, 0), (h_b, 2)):
                            for a in range(2):
                                c = pb + a
                                nc.sync.dma_start(
                                    out=st[32 * c : 32 * c + 32, :].rearrange(
                                        "i (b j) -> i b j", j=32
                                    ),
                                    in_=src[hh, a, :, 16 * fc : 16 * fc + 16, :],
                                )
                        st2 = stage.tile([P, QC], F32, tag="stage2")
                        nc.vector.transpose(st2[:], st[:])
                        ch = qkt_pool.tile([P, QC], F16, tag="qkT")
                        cast_eng.tensor_copy(ch[:], st2[:])
                        qkt[name].append(ch)
                qkt_all.append(qkt)

                v_aug = {}
                for hh, part in ((h_a, 0), (h_b, 1)):
                    vst = stage.tile([P, KT, D], F32, tag="vstage")
                    nc.sync.dma_start(out=vst[:], in_=v_v[hh])
                    va = vpool.tile([P, KT, D], BF16, tag="v")
                    cast_eng.tensor_copy(va[:], vst[:])
                    v_aug[part] = va
                v_all.append(v_aug)

            # ---------------- main (one flat pipeline over all pairs) -------
            # QK^T runs PIPE k-tiles ahead (crossing q-chunk and pair
            # boundaries), and each chunk's epilogue is interleaved into the
            # next chunk's k-tile loop late enough that every dependency is
            # satisfied before the owning engine reaches the instruction.
            TT = NPAIR * NQC * KT

            def emit_qkt(t):
                pr, rem = divmod(t, NQC * KT)
                gq, kt = divmod(rem, KT)
                s_ps = spsum.tile([P, 2, QC], F32, tag="s", name="s_ps")
                k_ch = qkt_all[pr]["k"][kt // 4]
                k_sl = slice((kt % 4) * P, (kt % 4 + 1) * P)
                for part, base in ((0, 0), (1, 64)):
                    nc.tensor.matmul(
                        s_ps[:, part, :],
                        k_ch[base : base + 64, k_sl],
                        qkt_all[pr]["q"][gq][base : base + 64, :],
                        tile_position=(base, 0),
                    )
                return s_ps

            def epilogue_copies(st):
                # PSUM -> SBUF eviction on ACT; frees the pv/zz banks
                st["o_sb"] = osb_pool.tile([P, QC], F32, tag="ot", name="o_sb")
                nc.scalar.copy(st["o_sb"][:], st["pv"][:])
                st["z_sb"] = zsb_pool.tile([33, QC], F32, tag="zt", name="z_sb")
                nc.scalar.copy(st["z_sb"][:], st["zz"][0:33, :])

            def epilogue_transposes(st):
                # transposed output + Z share one borrowed s-pool slot:
                # [128, 4, 161] f32 = 2576B <= the 4KB s slot
                ep = spsum.tile([P, 4, P + 33], F32, tag="s", name="ep")
                st["ep"] = ep
                for c in range(4):
                    nc.tensor.transpose(
                        ep[:, c, 0:P],
                        st["o_sb"][:, c * P : (c + 1) * P],
                        ident[:],
                    )
                for c in range(4):
                    nc.tensor.transpose(
                        ep[:, c, P : P + 33],
                        st["z_sb"][:, c * P : (c + 1) * P],
                        ident33[:],
                    )

            def epilogue_recip(st):
                rz = rz_pool.tile([P, 4, 2], F32, tag="rz", name="rz")
                st["rz"] = rz
                nc.vector.reciprocal(rz[:], st["ep"][:, :, P : P + 33 : 32])

            def epilogue_norm(st):
                ep, rz, g = st["ep"], st["rz"], st["g"]
                out_sb = outsb_pool.tile([P, 4, 2, D], F32, tag="out", name="out_sb")
                if NORM_ACT:
                    for c in range(4):
                        for hp in range(2):
                            nc.scalar.mul(
                                out_sb[:, c, hp, :],
                                ep[:, c, 64 * hp : 64 * hp + 64],
                                rz[:, c, hp],
                            )
                else:
                    ep0 = ep[:, :, 0:P].rearrange("p c (h d) -> p c h d", h=2)
                    nc.vector.tensor_tensor(
                        out=out_sb[:],
                        in0=ep0,
                        in1=rz[:].broadcast_to([P, 4, 2, D]),
                        op=mybir.AluOpType.mult,
                    )
                for part, hh in ((0, st["h_a"]), (1, st["h_b"])):
                    nc.sync.dma_start(
                        out=o_v[hh, :, g, :, :], in_=out_sb[:, :, part, :]
                    )

            s_tiles = {t: emit_qkt(t) for t in range(PIPE)}
            pend = None  # epilogue state of the previous q-chunk
            pv = zz = None
            for t in range(TT):
                pair, rem = divmod(t, NQC * KT)
                g, kt = divmod(rem, KT)
                v_aug = v_all[pair]
                if kt == 0:
                    pv = pv_pool.tile([P, QC], F32, tag="pv", name="pv")
                    zz = z_pool.tile([P, QC], F32, tag="z", name="zz")
                s_ps = s_tiles.pop(t)
                p_sb = ppool.tile([P, 2, QC], BF16, tag="p")
                if kt in _DVE_KTS:
                    nc.vector.tensor_scalar(
                        out=p_sb[:].bitcast(I16),
                        in0=s_ps[:],
                        scalar1=_SCH_A,
                        scalar2=_SCH_B,
                        op0=mybir.AluOpType.mult,
                        op1=mybir.AluOpType.add,
                    )
                else:
                    nc.scalar.activation(
                        p_sb[:], s_ps[:], mybir.ActivationFunctionType.Exp
                    )
                # col-packed PV pair: head A -> PSUM rows 0:64,
                # head B -> rows 64:128, concurrent on the PE
                for part, cb in ((0, 0), (1, 64)):
                    nc.tensor.matmul(
                        pv[cb : cb + 64, :],
                        v_aug[part][:, kt, :],
                        p_sb[:, part, :],
                        start=(kt == 0),
                        stop=(kt == KT - 1),
                        tile_position=(0, cb),
                    )
                # col-packed denominator pair: Z_A -> row 0, Z_B -> 32
                for part, cb in ((0, 0), (1, 32)):
                    nc.tensor.matmul(
                        zz[cb : cb + 1, :],
                        ones_w[:],
                        p_sb[:, part, :],
                        start=(kt == 0),
                        stop=(kt == KT - 1),
                        tile_position=(0, cb),
                    )
                if t + PIPE < TT:
                    s_tiles[t + PIPE] = emit_qkt(t + PIPE)
                if pend is not None:
                    if kt == 3:
                        epilogue_copies(pend)
                    elif kt == 5:
                        epilogue_transposes(pend)
                    elif kt == 7:
                        epilogue_recip(pend)
                    elif kt == 9:
                        epilogue_norm(pend)
                        pend = None
                if kt == KT - 1:
                    pend = {
                        "pv": pv,
                        "zz": zz,
                        "g": g,
                        "h_a": 2 * pair,
                        "h_b": 2 * pair + 1,
                    }
            # tail: epilogue of the very last q-chunk
            epilogue_copies(pend)
            epilogue_transposes(pend)
            epilogue_recip(pend)
            epilogue_norm(pend)
            pend = None
    nc.finalize()
    return nc


_NC_CACHE = None


def _get_nc():
    global _NC_CACHE
    if _NC_CACHE is None:
        _NC_CACHE = build_attention_nc()
    return _NC_CACHE


def kernel(q: np.ndarray, k: np.ndarray, v: np.ndarray) -> np.ndarray:
    q = np.asarray(q, dtype=np.float32)
    k = np.asarray(k, dtype=np.float32)
    v = np.asarray(v, dtype=np.float32)
    nc = _get_nc()
    in_maps = [
        {
            "q": np.ascontiguousarray(q[c * H : (c + 1) * H]),
            "k": np.ascontiguousarray(k[c * H : (c + 1) * H]),
            "v": np.ascontiguousarray(v[c * H : (c + 1) * H]),
        }
        for c in range(NCORES)
    ]
    res = run_bass_kernel_spmd(nc, in_maps, list(range(NCORES)))
    return np.concatenate([res.results[c]["out"] for c in range(NCORES)], axis=0)
